# revision 7
# baseline (speedup 1.0000x reference)
"""Trainium2 Bass kernel for nn_Encoder (bidirectional-LSTM encoder + attention).

Strategy: data-parallel over batch B=128 across 8 cores (16 batch elems/core).
Each core runs the full pipeline locally (embedding gather, input projections,
both LSTM directions for sentence+target, attention, output head). No
cross-core communication; host concatenates the per-core [16, 3] outputs.

All LSTM state is kept gate-transposed ([gate_dim, batch] with gate_dim on
partitions) so the elementwise gate math uses all 128 lanes. tanh is computed
via sigmoid (tanh(x) = 2*sigmoid(2x) - 1, with the g-gate weight rows
pre-scaled by 2 on the host) so the ACT table never switches mid-recurrence.
"""

import sys

sys.path.insert(0, "/opt/trn_rl_repo")

import numpy as np
import ml_dtypes

import concourse.bass as bass
import concourse.mybir as mybir
import concourse.tile as tile
from concourse.bass_utils import run_bass_kernel_spmd
from concourse.masks import make_identity
from concourse.vector_clock import ScopedClock

V, E, H, OUT = 100000, 300, 256, 3
B, LS, LT = 128, 128, 8
NCORES = 8
BL = B // NCORES  # 16 batch elements per core
G4 = 4 * H  # 1024 (gate dim)
KE = E + 1  # 301: embedding dim + bias row
NTOK_S = BL * LS  # 2048 sentence tokens per core
NTOK_T = BL * LT  # 128 target tokens per core

dt = mybir.dt
AF = mybir.ActivationFunctionType
ALU = mybir.AluOpType
f32 = dt.float32
bf16 = dt.bfloat16


# ---------------------------------------------------------------------------
# Workaround: this walrus build rejects >2 semaphore waits on one CTRL
# instruction; split the TileContext exit-drain waits onto individual nops.
def _patched_drain_and_barrier(self, tick_clock, wait_clock):
    nc = self.nc
    collect = nc.sync.nop()
    wait_clock.add_sem_waits(collect.ins, ScopedClock({None: tick_clock.global_clock}))
    si = collect.ins.sync_info
    waits = list(si.on_wait) if si and si.on_wait else []
    if len(waits) > 1:
        si.on_wait = waits[:1]
        for w in waits[1:]:
            nop = nc.sync.nop()
            if nop.ins.sync_info is None:
                nop.ins.sync_info = mybir.SyncInfo(on_wait=[w], on_update=[])
            else:
                nop.ins.sync_info.on_wait = [w]
    nc.sync.drain()
    nc.all_engine_barrier()
    popped = nc._tile_sem_poison_stack.pop()
    assert popped is self._sem_poison
    nc.clear_and_free_semaphores(list(self.sems.allocated().values()))
    nc.all_engine_barrier()


tile.TileContext._drain_and_barrier = _patched_drain_and_barrier


def _split_sync_waits(nc, max_waits=1):
    """Hoist excess semaphore waits (>max_waits per instruction) onto
    same-engine NoOp instructions inserted just before, preserving engine
    stream order (this walrus build encodes at most 2 waits/instruction)."""
    import bass_rust as _br

    ctr = [0]
    for fn in nc.m.functions:
        for bb in fn.blocks:
            out = []
            changed = False
            for inst in bb.instructions:
                si = getattr(inst, "sync_info", None)
                if si is not None and si.on_wait and len(si.on_wait) > max_waits:
                    waits = list(si.on_wait)
                    si.on_wait = waits[:max_waits]
                    rest = waits[max_waits:]
                    for j in range(0, len(rest), max_waits):
                        ctr[0] += 1
                        nop = _br.InstNoOp(name=f"WS-{ctr[0]}", ins=[], outs=[])
                        nop.engine = inst.engine
                        nop.sync_info = mybir.SyncInfo(
                            on_wait=rest[j : j + max_waits], on_update=[]
                        )
                        out.append(nop)
                    changed = True
                out.append(inst)
            if changed:
                bb.instructions = out


# ---------------------------------------------------------------------------


def _emit_lstm(nc, pools, T, d, xw_v, whh, hsT_v, first_tag):
    """Emit one LSTM direction's recurrence.

    T: sequence length. d: 0=forward, 1=backward (within this LSTM's hsT).
    xw_v:   [128, 8, BL, T] bf16 view of precomputed input projections (+bias).
    whh:    [128, 2, 1024] bf16 recurrent weights (lhsT tiles).
    hsT_v:  [128, 4, BL, T] bf16 view; this direction writes slots 2d, 2d+1.
    """
    spool, pgates, hzero = pools
    CH = 4  # psum chunk: 4 timesteps per bank
    gtag = f"g_{first_tag[-1]}"  # share psum slots between tgt/sen per direction
    ctag = f"c_{first_tag}"
    steps = range(T - 1, -1, -1) if d == 1 else range(T)
    c_prev = None
    for si, t in enumerate(steps):
        tl = si % CH
        if tl == 0:
            g = pgates.tile([128, CH * 128], f32, tag=gtag, name=f"g_{first_tag}_{si}")
        gv = g[:, tl * 128 : (tl + 1) * 128]
        gv3 = gv.rearrange("p (m b) -> p m b", b=BL)
        # recurrent matmuls: gates += Whh.T-tiles @ h_prev
        for m in range(8):
            for k in range(2):
                if si == 0:
                    h_prev = hzero[:, k, :]
                else:
                    h_prev = hsT_v[:, 2 * d + k, :, t + (1 if d == 1 else -1)]
                nc.tensor.matmul(
                    gv3[:, m, :],
                    whh[:, k, m * 128 : (m + 1) * 128],
                    h_prev,
                    start=(k == 0),
                    stop=(k == 1),
                )
        # add input projection (in-place in psum)
        nc.vector.tensor_tensor(gv3, gv3, xw_v[:, :, :, t], op=ALU.add)
        # all four gates through sigmoid in one op (g-gate pre-scaled by 2)
        sig = spool.tile([128, 128], bf16, tag=f"sig_{first_tag}", name=f"sig_{first_tag}_{si}")
        nc.scalar.activation(sig, gv, AF.Sigmoid)
        # u = tanh(g) = 2*sig(2g) - 1
        u = spool.tile([128, 32], bf16, tag=f"u_{first_tag}", name=f"u_{first_tag}_{si}")
        nc.vector.tensor_scalar(u, sig[:, 64:96], 2.0, -1.0, ALU.mult, ALU.add)
        cn = spool.tile([128, 32], f32, tag=ctag, name=f"c_{first_tag}_{si}")
        if si == 0:
            nc.vector.tensor_tensor(cn, sig[:, 0:32], u, op=ALU.mult)
        else:
            t2 = spool.tile([128, 32], f32, tag=f"t2_{first_tag}", name=f"t2_{first_tag}_{si}")
            m1 = spool.tile([128, 32], f32, tag=f"m1_{first_tag}", name=f"m1_{first_tag}_{si}")
            nc.vector.tensor_tensor(t2, sig[:, 32:64], c_prev, op=ALU.mult)
            nc.vector.tensor_tensor(m1, sig[:, 0:32], u, op=ALU.mult)
            nc.vector.tensor_tensor(cn, t2, m1, op=ALU.add)
        c_prev = cn
        # h = sig(o) * tanh(c)
        vt = spool.tile([128, 32], bf16, tag=f"v_{first_tag}", name=f"v_{first_tag}_{si}")
        nc.scalar.activation(vt, cn, AF.Sigmoid, scale=2.0)
        w1 = spool.tile([128, 32], bf16, tag=f"w1_{first_tag}", name=f"w1_{first_tag}_{si}")
        nc.vector.tensor_scalar(w1, vt, 2.0, -1.0, ALU.mult, ALU.add)
        nc.vector.tensor_tensor(
            hsT_v[:, 2 * d : 2 * d + 2, :, t], w1.rearrange("p (s b) -> p s b", b=BL),
            sig[:, 96:128].rearrange("p (s b) -> p s b", b=BL), op=ALU.mult,
        )


def _build_program():
    nc = bass.Bass("TRN2", target_bir_lowering=False, debug=False)

    # --- DRAM I/O -----------------------------------------------------------
    d_emb = nc.dram_tensor("emb", [V, E], f32, kind="ExternalInput").ap()
    d_sidx = nc.dram_tensor("sen_idx", [128, NTOK_S // 128], dt.int32, kind="ExternalInput").ap()
    d_tidx = nc.dram_tensor("tgt_idx", [128, 1], dt.int32, kind="ExternalInput").ap()
    d_wih = {}
    d_whh = {}
    for nm in ("sf", "sb", "tf", "tb"):
        d_wih[nm] = nc.dram_tensor(f"wih_{nm}", [3, 128, G4], bf16, kind="ExternalInput").ap()
        d_whh[nm] = None
        d_whh[nm] = nc.dram_tensor(f"whh_{nm}", [2, 128, G4], bf16, kind="ExternalInput").ap()
    d_bias = {
        nm: nc.dram_tensor(f"bias_{nm}", [128, 8], f32, kind="ExternalInput").ap()
        for nm in ("sf", "sb", "tf", "tb")
    }
    d_wout = nc.dram_tensor("woutT", [4, 128, OUT], bf16, kind="ExternalInput").ap()
    d_bout = nc.dram_tensor("boutT", [OUT, 1], f32, kind="ExternalInput").ap()
    d_out = nc.dram_tensor("out", [BL, OUT], f32, kind="ExternalOutput").ap()

    with tile.TileContext(nc) as tc:
        with (
            tc.tile_pool(name="cpool", bufs=1) as cpool,
            tc.tile_pool(name="spool", bufs=2) as spool,
            tc.tile_pool(name="ptr", bufs=2, space="PSUM") as ptr,
            tc.tile_pool(name="pproj", bufs=2, space="PSUM") as pproj,
            tc.tile_pool(name="pgates", bufs=2, space="PSUM") as pgates,
        ):
            # --- constants / weights into SBUF ------------------------------
            wih = {}
            whh = {}
            bias_sb = {}
            for nm in ("sf", "sb", "tf", "tb"):
                wt = cpool.tile([128, 3, G4], bf16, name=f"wih_{nm}_sb")
                nc.sync.dma_start(wt, d_wih[nm].rearrange("k p m -> p k m"))
                wih[nm] = wt
                ht = cpool.tile([128, 2, G4], bf16, name=f"whh_{nm}_sb")
                nc.sync.dma_start(ht, d_whh[nm].rearrange("k p m -> p k m"))
                whh[nm] = ht
                bt = cpool.tile([128, 8], f32, name=f"bias_{nm}_sb")
                nc.sync.dma_start(bt, d_bias[nm])
                bias_sb[nm] = bt
            wout_sb = cpool.tile([128, 4, OUT], bf16, name="wout_sb")
            nc.sync.dma_start(wout_sb, d_wout.rearrange("k p m -> p k m"))
            bout_sb = cpool.tile([OUT, 1], f32, name="bout_sb")
            nc.sync.dma_start(bout_sb, d_bout)
            sidx = cpool.tile([128, NTOK_S // 128], dt.int32, name="sidx")
            nc.sync.dma_start(sidx, d_sidx)
            tidx = cpool.tile([128, 1], dt.int32, name="tidx")
            nc.sync.dma_start(tidx, d_tidx)

            ident = cpool.tile([128, 128], f32, name="ident")
            make_identity(nc, ident)
            ones = cpool.tile([128, 128], f32, name="ones")
            nc.gpsimd.memset(ones, 1.0)
            hzero = cpool.tile([128, 2, BL], bf16, name="hzero")
            nc.vector.memset(hzero, 0.0)

            # --- persistent activations -------------------------------------
            xT_s = cpool.tile([128, 3, NTOK_S], bf16, name="xT_s")
            xT_t = cpool.tile([128, 3, NTOK_T], bf16, name="xT_t")
            xw = {
                "sf": cpool.tile([128, 8, NTOK_S], bf16, name="xw_sf"),
                "sb": cpool.tile([128, 8, NTOK_S], bf16, name="xw_sb"),
                "tf": cpool.tile([128, 8, NTOK_T], bf16, name="xw_tf"),
                "tb": cpool.tile([128, 8, NTOK_T], bf16, name="xw_tb"),
            }
            hsT = cpool.tile([128, 4, NTOK_S], bf16, name="hsT")
            ttT = cpool.tile([128, 4, NTOK_T], bf16, name="ttT")

            # --- phase 1: gather + transpose --------------------------------
            def gather_and_transpose(idx_tile, n_tiles, xT, which):
                for j in range(n_tiles):
                    gx = spool.tile([128, E], f32, tag="gx", name=f"gx_{which}_{j}")
                    nc.gpsimd.indirect_dma_start(
                        out=gx,
                        out_offset=None,
                        in_=d_emb[:, :],
                        in_offset=bass.IndirectOffsetOnAxis(ap=idx_tile[:, j : j + 1], axis=0),
                    )
                    for k in range(3):
                        kn = 128 if k < 2 else E - 256
                        pt = ptr.tile([128, 128], f32, tag="pt", name=f"pt_{which}_{j}_{k}")
                        nc.tensor.transpose(pt[0:kn, :], gx[:, k * 128 : k * 128 + kn], ident)
                        eng = nc.scalar if (j + k) % 2 == 0 else nc.vector
                        if eng is nc.scalar:
                            nc.scalar.activation(xT[0:kn, k, j * 128 : (j + 1) * 128], pt[0:kn, :], AF.Copy)
                        else:
                            nc.vector.tensor_copy(xT[0:kn, k, j * 128 : (j + 1) * 128], pt[0:kn, :])

            gather_and_transpose(sidx, NTOK_S // 128, xT_s, "s")
            gather_and_transpose(tidx, NTOK_T // 128, xT_t, "t")

            # --- phase 1b: input projections --------------------------------
            def proj(xT, ntok, names):
                xT_v = xT.rearrange("p k (b t) -> p k b t", b=BL)
                T = ntok // BL
                CT = min(32, T)  # timesteps per chunk -> N = BL*CT <= 512
                for nm in names:
                    xw_v = xw[nm].rearrange("p m (b t) -> p m b t", b=BL)
                    for c in range(T // CT):
                        tsl = slice(c * CT, (c + 1) * CT)
                        for m in range(8):
                            pp = pproj.tile([128, 512], f32, tag="pp", name=f"pp_{nm}_{c}_{m}")
                            po = pp[:, : BL * CT].rearrange("p (b t) -> p b t", b=BL)
                            for k in range(3):
                                kn = 128 if k < 2 else 44
                                nc.tensor.matmul(
                                    po,
                                    wih[nm][0:kn, k, m * 128 : (m + 1) * 128],
                                    xT_v[0:kn, k, :, tsl],
                                    start=(k == 0),
                                    stop=(k == 2),
                                )
                            if m % 2 == 0:
                                nc.scalar.activation(
                                    xw_v[:, m, :, tsl], po, AF.Identity,
                                    bias=bias_sb[nm][:, m : m + 1],
                                )
                            else:
                                nc.vector.tensor_scalar(
                                    xw_v[:, m, :, tsl], po, bias_sb[nm][:, m : m + 1],
                                    None, op0=ALU.add,
                                )

            proj(xT_t, NTOK_T, ("tf", "tb"))
            proj(xT_s, NTOK_S, ("sf", "sb"))

            # --- phase 2: recurrences ---------------------------------------
            lpools = (spool, pgates, hzero)
            ttT_v = ttT.rearrange("p s (b t) -> p s b t", b=BL)
            _emit_lstm(nc, lpools, LT, 0, xw["tf"].rearrange("p m (b t) -> p m b t", b=BL), whh["tf"], ttT_v, "tf")
            _emit_lstm(nc, lpools, LT, 1, xw["tb"].rearrange("p m (b t) -> p m b t", b=BL), whh["tb"], ttT_v, "tb")
            hsT_v = hsT.rearrange("p s (b t) -> p s b t", b=BL)
            _emit_lstm(nc, lpools, LS, 0, xw["sf"].rearrange("p m (b t) -> p m b t", b=BL), whh["sf"], hsT_v, "sf")
            _emit_lstm(nc, lpools, LS, 1, xw["sb"].rearrange("p m (b t) -> p m b t", b=BL), whh["sb"], hsT_v, "sb")

        # --- phase 3: attention + output head ------------------------------
        with (
            tc.tile_pool(name="apool", bufs=1) as apool,
            tc.tile_pool(name="patt", bufs=1, space="PSUM") as patt,
        ):
            # A[b,s,t] stored as [s(part), b*8+t]
            a3 = patt.tile([128, 128], f32, name="a3")
            for b in range(BL):
                for k in range(4):
                    nc.tensor.matmul(
                        a3[:, b * 8 : (b + 1) * 8],
                        hsT[:, k, b * 128 : (b + 1) * 128],
                        ttT[:, k, b * 8 : (b + 1) * 8],
                        start=(k == 0),
                        stop=(k == 3),
                    )
            expA = apool.tile([128, 128], f32, name="expA")
            nc.scalar.activation(expA, a3, AF.Exp)
            expA_v = expA.rearrange("p (b t) -> p b t", t=LT)
            # row softmax (over t) then mean over s, divided by col sums (over s)
            rsum = apool.tile([128, BL], f32, name="rsum")
            nc.vector.tensor_reduce(rsum, expA_v, axis=mybir.AxisListType.X, op=ALU.add)
            rr = apool.tile([128, BL], f32, name="rr")
            nc.vector.reciprocal(rr, rsum)
            rnorm = apool.tile([128, 128], f32, name="rnorm")
            rr_b = bass.AP(tensor=rr.tensor, offset=rr.offset, ap=list(rr.ap) + [[0, LT]])
            nc.vector.tensor_tensor(rnorm.rearrange("p (b t) -> p b t", t=LT), expA_v, rr_b, op=ALU.mult)
            rvp = patt.tile([1, 128], f32, name="rvp")
            nc.tensor.matmul(rvp, ones[:, 0:1], rnorm, start=True, stop=True)
            csum = patt.tile([1, 128], f32, name="csum")
            nc.tensor.matmul(csum, ones[:, 0:1], expA, start=True, stop=True)
            rc = apool.tile([1, 128], f32, name="rc")
            nc.vector.reciprocal(rc, csum)
            q = apool.tile([1, 128], f32, name="q")
            nc.vector.scalar_tensor_tensor(q, rvp, 1.0 / LS, rc, op0=ALU.mult, op1=ALU.mult)
            qbc = patt.tile([128, 128], f32, name="qbc")
            nc.tensor.matmul(qbc, ones[0:1, :], q, start=True, stop=True)
            attw = apool.tile([128, 128], f32, name="attw")
            nc.vector.tensor_tensor(attw, expA, qbc, op=ALU.mult)
            attnT = apool.tile([128, BL], f32, name="attnT")
            nc.vector.tensor_reduce(attnT, attw.rearrange("p (b t) -> p b t", t=LT), axis=mybir.AxisListType.X, op=ALU.add)
            attnb = apool.tile([128, BL], bf16, name="attnb")
            nc.vector.tensor_copy(attnb, attnT)

            # de-transpose sen_h for the score contraction
            sen_h = apool.tile([128, BL, 4 * 128], bf16, name="sen_h")
            for b in range(BL):
                for k in range(4):
                    nc.sync.dma_start_transpose(
                        sen_h[:, b, k * 128 : (k + 1) * 128], hsT[:, k, b * 128 : (b + 1) * 128]
                    )
            scoT = patt.tile([128, 4 * BL], f32, name="scoT")
            for b in range(BL):
                for mh in range(4):
                    nc.tensor.matmul(
                        scoT[:, b * 4 + mh : b * 4 + mh + 1],
                        sen_h[:, b, mh * 128 : (mh + 1) * 128],
                        attnb[:, b : b + 1],
                        start=True,
                        stop=True,
                    )
            scoB = apool.tile([128, 4 * BL], bf16, name="scoB")
            nc.scalar.activation(scoB, scoT, AF.Copy)
            lgT = patt.tile([OUT, BL], f32, name="lgT")
            for mh in range(4):
                nc.tensor.matmul(
                    lgT, wout_sb[:, mh, :], scoB[:, mh :: 4], start=(mh == 0), stop=(mh == 3)
                )
            lgsb = apool.tile([OUT, BL], f32, name="lgsb")
            nc.scalar.activation(lgsb, lgT, AF.Identity, bias=bout_sb[0:OUT, 0:1])
            lg2 = patt.tile([BL, OUT], f32, name="lg2")
            nc.tensor.transpose(lg2, lgsb, ident[0:OUT, 0:OUT])
            eo = apool.tile([BL, OUT], f32, name="eo")
            nc.scalar.activation(eo, lg2, AF.Exp)
            es = apool.tile([BL, 1], f32, name="es")
            nc.vector.tensor_reduce(es, eo, axis=mybir.AxisListType.X, op=ALU.add)
            er = apool.tile([BL, 1], f32, name="er")
            nc.vector.reciprocal(er, es)
            res = apool.tile([BL, OUT], f32, name="res")
            nc.vector.tensor_scalar(res, eo, er, None, op0=ALU.mult)
            nc.sync.dma_start(d_out, res)

    _split_sync_waits(nc)
    return nc


_CACHE = {}


def _get_program():
    if "nc" not in _CACHE:
        _CACHE["nc"] = _build_program()
    return _CACHE["nc"]


def prepare_in_maps(inputs):
    """Host-side prep: shard + repack inputs into per-core in_maps."""
    bf = ml_dtypes.bfloat16
    sen = np.asarray(inputs["sentence_source"]).astype(np.int32)  # [B, LS]
    tgt = np.asarray(inputs["target_source"]).astype(np.int32)  # [B, LT]
    emb = np.ascontiguousarray(np.asarray(inputs["emb_W"], dtype=np.float32))
    emb = emb.copy()
    emb[0, :] = 0.0  # padding_idx

    def pack_wih(nm):
        W = np.asarray(inputs[f"Wih_{nm}"], dtype=np.float32).T.copy()  # [300, 1024]
        W[:, 2 * H : 3 * H] *= 2.0  # g-gate: tanh via sigmoid
        pack = np.zeros((3, 128, G4), dtype=bf)
        pack[0] = W[0:128].astype(bf)
        pack[1] = W[128:256].astype(bf)
        pack[2, 0:44] = W[256:300].astype(bf)
        return pack

    def pack_bias(nm):
        bias = (
            np.asarray(inputs[f"bih_{nm}"], dtype=np.float32)
            + np.asarray(inputs[f"bhh_{nm}"], dtype=np.float32)
        ).copy()
        bias[2 * H : 3 * H] *= 2.0
        return np.ascontiguousarray(bias.reshape(8, 128).T)

    def pack_whh(nm):
        W = np.asarray(inputs[f"Whh_{nm}"], dtype=np.float32).T.copy()  # [256, 1024]
        W[:, 2 * H : 3 * H] *= 2.0
        return np.ascontiguousarray(W.reshape(2, 128, G4).astype(bf))

    shared = {"emb": emb}
    for nm in ("sf", "sb", "tf", "tb"):
        shared[f"wih_{nm}"] = pack_wih(nm)
        shared[f"bias_{nm}"] = pack_bias(nm)
        shared[f"whh_{nm}"] = pack_whh(nm)
    Wout = np.asarray(inputs["Wout"], dtype=np.float32)  # [3, 512]
    shared["woutT"] = np.ascontiguousarray(Wout.T.reshape(4, 128, OUT).astype(bf))
    shared["boutT"] = np.asarray(inputs["bout"], dtype=np.float32).reshape(OUT, 1)

    in_maps = []
    for c in range(NCORES):
        sl = slice(c * BL, (c + 1) * BL)
        m = dict(shared)
        m["sen_idx"] = np.ascontiguousarray(sen[sl].reshape(NTOK_S // 128, 128).T)
        m["tgt_idx"] = np.ascontiguousarray(tgt[sl].reshape(NTOK_T // 128, 128).T)
        in_maps.append(m)
    return in_maps


def kernel(**inputs) -> np.ndarray:
    nc = _get_program()
    in_maps = prepare_in_maps(inputs)
    r = run_bass_kernel_spmd(nc, in_maps, core_ids=list(range(NCORES)))
    return np.concatenate([r.results[c]["out"] for c in range(NCORES)], axis=0)


if __name__ == "__main__":
    print("building program...")
    nc = _get_program()
    print("build OK")


# revision 12
# speedup vs baseline: 71.8728x; 71.8728x over previous
"""Trainium2 Bass kernel for nn_Encoder (bidirectional-LSTM encoder + attention).

Strategy: data-parallel over batch B=128 across 8 cores (16 batch elems/core).
Each core runs the full pipeline locally (embedding gather, input projections,
both LSTM directions for sentence+target, attention, output head). No
cross-core communication; host concatenates the per-core [16, 3] outputs.

All LSTM state is kept gate-transposed ([gate_dim, batch] with gate_dim on
partitions) so the elementwise gate math uses all 128 lanes. tanh is computed
via sigmoid (tanh(x) = 2*sigmoid(2x) - 1, with the g-gate weight rows
pre-scaled by 2 on the host) so the ACT table never switches mid-recurrence.
"""

import sys

sys.path.insert(0, "/opt/trn_rl_repo")

import numpy as np
import ml_dtypes

import concourse.bass as bass
import concourse.mybir as mybir
import concourse.tile as tile
from concourse.bass_utils import run_bass_kernel_spmd
from concourse.masks import make_identity
from concourse.vector_clock import ScopedClock

V, E, H, OUT = 100000, 300, 256, 3
B, LS, LT = 128, 128, 8
NCORES = 8
BL = B // NCORES  # 16 batch elements per core
G4 = 4 * H  # 1024 (gate dim)
KE = E + 1  # 301: embedding dim + bias row
NTOK_S = BL * LS  # 2048 sentence tokens per core
NTOK_T = BL * LT  # 128 target tokens per core

dt = mybir.dt
AF = mybir.ActivationFunctionType
ALU = mybir.AluOpType
f32 = dt.float32
bf16 = dt.bfloat16


# ---------------------------------------------------------------------------
# Workaround: this walrus build rejects >2 semaphore waits on one CTRL
# instruction; split the TileContext exit-drain waits onto individual nops.
def _patched_drain_and_barrier(self, tick_clock, wait_clock):
    nc = self.nc
    collect = nc.sync.nop()
    wait_clock.add_sem_waits(collect.ins, ScopedClock({None: tick_clock.global_clock}))
    si = collect.ins.sync_info
    waits = list(si.on_wait) if si and si.on_wait else []
    if len(waits) > 1:
        si.on_wait = waits[:1]
        for w in waits[1:]:
            nop = nc.sync.nop()
            if nop.ins.sync_info is None:
                nop.ins.sync_info = mybir.SyncInfo(on_wait=[w], on_update=[])
            else:
                nop.ins.sync_info.on_wait = [w]
    nc.sync.drain()
    nc.all_engine_barrier()
    popped = nc._tile_sem_poison_stack.pop()
    assert popped is self._sem_poison
    nc.clear_and_free_semaphores(list(self.sems.allocated().values()))
    nc.all_engine_barrier()


tile.TileContext._drain_and_barrier = _patched_drain_and_barrier


def _split_sync_waits(nc, max_waits=1):
    """Hoist excess semaphore waits (>max_waits per instruction) onto
    same-engine NoOp instructions inserted just before, preserving engine
    stream order (this walrus build encodes at most 2 waits/instruction)."""
    import bass_rust as _br

    ctr = [0]
    for fn in nc.m.functions:
        for bb in fn.blocks:
            out = []
            changed = False
            for inst in bb.instructions:
                si = getattr(inst, "sync_info", None)
                if si is not None and si.on_wait and len(si.on_wait) > max_waits:
                    waits = list(si.on_wait)
                    si.on_wait = waits[:max_waits]
                    rest = waits[max_waits:]
                    for j in range(0, len(rest), max_waits):
                        ctr[0] += 1
                        nop = _br.InstNoOp(name=f"WS-{ctr[0]}", ins=[], outs=[])
                        nop.engine = inst.engine
                        nop.sync_info = mybir.SyncInfo(
                            on_wait=rest[j : j + max_waits], on_update=[]
                        )
                        out.append(nop)
                    changed = True
                out.append(inst)
            if changed:
                bb.instructions = out


# ---------------------------------------------------------------------------


def _emit_lstm(nc, pools, T, d, xw_v, whh, hsT_v, first_tag):
    """Emit one LSTM direction's recurrence (v2: independent per-direction
    chains; fwd/bwd interleave on the engines via the Tile scheduler).

    T: sequence length. d: 0=forward, 1=backward (within this LSTM's hsT).
    xw_v:   [128, 8, BL, T] bf16 view of precomputed input projections (+bias).
    whh:    [128, 2, 1024] bf16 recurrent weights (lhsT tiles), host-scaled x2
            (hidden states are stored halved).
    hsT_v:  [128, 4, BL, T] bf16 view; this direction writes slots 2d, 2d+1
            holding h/2 (downstream consumers compensate).
    """
    spool, pgates, hzero, ibf = pools
    CH = 4  # psum chunk: 4 timesteps per bank
    gtag = f"g_{first_tag[-1]}"  # share psum slots between tgt/sen per direction
    ctag = f"c_{first_tag}"
    steps = range(T - 1, -1, -1) if d == 1 else range(T)
    c_prev = None
    for si, t in enumerate(steps):
        tl = si % CH
        if tl == 0:
            g = pgates.tile([128, CH * 128], f32, tag=gtag, name=f"g_{first_tag}_{si}")
        gv = g[:, tl * 128 : (tl + 1) * 128]
        gv3 = gv.rearrange("p (m b) -> p m b", b=BL)
        # inject input projection via identity matmul (sets has_written),
        # then accumulate the recurrent matmuls on top
        nc.tensor.matmul(gv3, ibf, xw_v[:, :, :, t], start=True, stop=False)
        for m in range(8):
            for k in range(2):
                if si == 0:
                    h_prev = hzero[:, k, :]
                else:
                    h_prev = hsT_v[:, 2 * d + k, :, t + (1 if d == 1 else -1)]
                nc.tensor.matmul(
                    gv3[:, m, :],
                    whh[:, k, m * 128 : (m + 1) * 128],
                    h_prev,
                    start=False,
                    stop=(m == 7 and k == 1),
                )
        # all four gates through sigmoid in one op (g-gate pre-scaled by 2)
        sig = spool.tile([128, 128], bf16, tag=f"sig_{first_tag}", name=f"sig_{first_tag}_{si}")
        nc.scalar.activation(sig, gv, AF.Sigmoid)
        # c = sig(f)*c + sig(i)*tanh(g);  tanh(g) = 2*sig(2g)-1, so
        # mh := (sig(2g)-0.5)*sig(i) = sig(i)*tanh(g)/2
        cn = spool.tile([128, 32], f32, tag=ctag, name=f"c_{first_tag}_{si}")
        mh = spool.tile([128, 32], f32, tag=f"mh_{first_tag}", name=f"mh_{first_tag}_{si}")
        nc.vector.scalar_tensor_tensor(
            mh, sig[:, 64:96], -0.5, sig[:, 0:32], op0=ALU.add, op1=ALU.mult
        )
        if si == 0:
            nc.vector.tensor_scalar(cn, mh, 2.0, None, op0=ALU.mult)
        else:
            t2 = spool.tile([128, 32], f32, tag=f"t2_{first_tag}", name=f"t2_{first_tag}_{si}")
            nc.vector.tensor_tensor(t2, sig[:, 32:64], c_prev, op=ALU.mult)
            nc.vector.scalar_tensor_tensor(cn, mh, 2.0, t2, op0=ALU.mult, op1=ALU.add)
        c_prev = cn
        # stored h/2 = (sig(2c)-0.5) * sig(o)  [= sig(o)*tanh(c)/2]
        vt = spool.tile([128, 32], bf16, tag=f"v_{first_tag}", name=f"v_{first_tag}_{si}")
        nc.scalar.activation(vt, cn, AF.Sigmoid, scale=2.0)
        nc.vector.scalar_tensor_tensor(
            hsT_v[:, 2 * d : 2 * d + 2, :, t],
            vt.rearrange("p (s b) -> p s b", b=BL), -0.5,
            sig[:, 96:128].rearrange("p (s b) -> p s b", b=BL),
            op0=ALU.add, op1=ALU.mult,
        )


def _build_program():
    nc = bass.Bass("TRN2", target_bir_lowering=False, debug=False)

    # --- DRAM I/O -----------------------------------------------------------
    d_emb = nc.dram_tensor("emb", [V, E], f32, kind="ExternalInput").ap()
    d_sidx = nc.dram_tensor("sen_idx", [128, NTOK_S // 128], dt.int32, kind="ExternalInput").ap()
    d_tidx = nc.dram_tensor("tgt_idx", [128, 1], dt.int32, kind="ExternalInput").ap()
    d_wih = {}
    d_whh = {}
    for nm in ("sf", "sb", "tf", "tb"):
        d_wih[nm] = nc.dram_tensor(f"wih_{nm}", [3, 128, G4], bf16, kind="ExternalInput").ap()
        d_whh[nm] = None
        d_whh[nm] = nc.dram_tensor(f"whh_{nm}", [2, 128, G4], bf16, kind="ExternalInput").ap()
    d_bias = {
        nm: nc.dram_tensor(f"bias_{nm}", [128, 8], f32, kind="ExternalInput").ap()
        for nm in ("sf", "sb", "tf", "tb")
    }
    d_wout = nc.dram_tensor("woutT", [4, 128, OUT], bf16, kind="ExternalInput").ap()
    d_bout = nc.dram_tensor("boutT", [OUT, 1], f32, kind="ExternalInput").ap()
    d_out = nc.dram_tensor("out", [BL, OUT], f32, kind="ExternalOutput").ap()

    with tile.TileContext(nc) as tc:
        with (
            tc.tile_pool(name="cpool", bufs=1) as cpool,
            tc.tile_pool(name="spool", bufs=2) as spool,
            tc.tile_pool(name="ptr", bufs=2, space="PSUM") as ptr,
            tc.tile_pool(name="pproj", bufs=2, space="PSUM") as pproj,
            tc.tile_pool(name="pgates", bufs=2, space="PSUM") as pgates,
        ):
            # --- constants / weights into SBUF ------------------------------
            wih = {}
            whh = {}
            bias_sb = {}
            for nm in ("sf", "sb", "tf", "tb"):
                wt = cpool.tile([128, 3, G4], bf16, name=f"wih_{nm}_sb")
                nc.sync.dma_start(wt, d_wih[nm].rearrange("k p m -> p k m"))
                wih[nm] = wt
                ht = cpool.tile([128, 2, G4], bf16, name=f"whh_{nm}_sb")
                nc.sync.dma_start(ht, d_whh[nm].rearrange("k p m -> p k m"))
                whh[nm] = ht
                bt = cpool.tile([128, 8], f32, name=f"bias_{nm}_sb")
                nc.sync.dma_start(bt, d_bias[nm])
                bias_sb[nm] = bt
            wout_sb = cpool.tile([128, 4, OUT], bf16, name="wout_sb")
            nc.sync.dma_start(wout_sb, d_wout.rearrange("k p m -> p k m"))
            bout_sb = cpool.tile([OUT, 1], f32, name="bout_sb")
            nc.sync.dma_start(bout_sb, d_bout)
            sidx = cpool.tile([128, NTOK_S // 128], dt.int32, name="sidx")
            nc.sync.dma_start(sidx, d_sidx)
            tidx = cpool.tile([128, 1], dt.int32, name="tidx")
            nc.sync.dma_start(tidx, d_tidx)

            ident = cpool.tile([128, 128], f32, name="ident")
            make_identity(nc, ident)
            ibf = cpool.tile([128, 128], bf16, name="ibf")
            make_identity(nc, ibf)
            ones = cpool.tile([128, 128], f32, name="ones")
            nc.gpsimd.memset(ones, 1.0)
            hzero = cpool.tile([128, 2, BL], bf16, name="hzero")
            nc.vector.memset(hzero, 0.0)

            # --- persistent activations -------------------------------------
            xT_s = cpool.tile([128, 3, NTOK_S], bf16, name="xT_s")
            xT_t = cpool.tile([128, 3, NTOK_T], bf16, name="xT_t")
            xw = {
                "sf": cpool.tile([128, 8, NTOK_S], bf16, name="xw_sf"),
                "sb": cpool.tile([128, 8, NTOK_S], bf16, name="xw_sb"),
                "tf": cpool.tile([128, 8, NTOK_T], bf16, name="xw_tf"),
                "tb": cpool.tile([128, 8, NTOK_T], bf16, name="xw_tb"),
            }
            hsT = cpool.tile([128, 4, NTOK_S], bf16, name="hsT")
            ttT = cpool.tile([128, 4, NTOK_T], bf16, name="ttT")

            # --- phase 1: gather + transpose --------------------------------
            def gather_and_transpose(idx_tile, n_tiles, xT, which):
                for j in range(n_tiles):
                    gx = spool.tile([128, E], f32, tag="gx", name=f"gx_{which}_{j}")
                    nc.gpsimd.indirect_dma_start(
                        out=gx,
                        out_offset=None,
                        in_=d_emb[:, :],
                        in_offset=bass.IndirectOffsetOnAxis(ap=idx_tile[:, j : j + 1], axis=0),
                    )
                    for k in range(3):
                        kn = 128 if k < 2 else E - 256
                        pt = ptr.tile([128, 128], f32, tag="pt", name=f"pt_{which}_{j}_{k}")
                        nc.tensor.transpose(pt[0:kn, :], gx[:, k * 128 : k * 128 + kn], ident)
                        eng = nc.scalar if (j + k) % 2 == 0 else nc.vector
                        if eng is nc.scalar:
                            nc.scalar.activation(xT[0:kn, k, j * 128 : (j + 1) * 128], pt[0:kn, :], AF.Copy)
                        else:
                            nc.vector.tensor_copy(xT[0:kn, k, j * 128 : (j + 1) * 128], pt[0:kn, :])

            gather_and_transpose(sidx, NTOK_S // 128, xT_s, "s")
            gather_and_transpose(tidx, NTOK_T // 128, xT_t, "t")

            # --- phase 1b: input projections --------------------------------
            def proj(xT, ntok, names):
                xT_v = xT.rearrange("p k (b t) -> p k b t", b=BL)
                T = ntok // BL
                CT = min(32, T)  # timesteps per chunk -> N = BL*CT <= 512
                for nm in names:
                    xw_v = xw[nm].rearrange("p m (b t) -> p m b t", b=BL)
                    for c in range(T // CT):
                        tsl = slice(c * CT, (c + 1) * CT)
                        for m in range(8):
                            pp = pproj.tile([128, 512], f32, tag="pp", name=f"pp_{nm}_{c}_{m}")
                            po = pp[:, : BL * CT].rearrange("p (b t) -> p b t", b=BL)
                            for k in range(3):
                                kn = 128 if k < 2 else 44
                                nc.tensor.matmul(
                                    po,
                                    wih[nm][0:kn, k, m * 128 : (m + 1) * 128],
                                    xT_v[0:kn, k, :, tsl],
                                    start=(k == 0),
                                    stop=(k == 2),
                                )
                            if m % 2 == 0:
                                nc.scalar.activation(
                                    xw_v[:, m, :, tsl], po, AF.Identity,
                                    bias=bias_sb[nm][:, m : m + 1],
                                )
                            else:
                                nc.vector.tensor_scalar(
                                    xw_v[:, m, :, tsl], po, bias_sb[nm][:, m : m + 1],
                                    None, op0=ALU.add,
                                )

            proj(xT_t, NTOK_T, ("tf", "tb"))
            proj(xT_s, NTOK_S, ("sf", "sb"))

            # --- phase 2: recurrences ---------------------------------------
            lpools = (spool, pgates, hzero, ibf)
            xwv = lambda nm: xw[nm].rearrange("p m (b t) -> p m b t", b=BL)
            ttT_v = ttT.rearrange("p s (b t) -> p s b t", b=BL)
            _emit_lstm(nc, lpools, LT, 0, xwv("tf"), whh["tf"], ttT_v, "tf")
            _emit_lstm(nc, lpools, LT, 1, xwv("tb"), whh["tb"], ttT_v, "tb")
            hsT_v = hsT.rearrange("p s (b t) -> p s b t", b=BL)
            _emit_lstm(nc, lpools, LS, 0, xwv("sf"), whh["sf"], hsT_v, "sf")
            _emit_lstm(nc, lpools, LS, 1, xwv("sb"), whh["sb"], hsT_v, "sb")

        # --- phase 3: attention + output head ------------------------------
        with (
            tc.tile_pool(name="apool", bufs=1) as apool,
            tc.tile_pool(name="patt", bufs=1, space="PSUM") as patt,
        ):
            # A[b,s,t] stored as [s(part), b*8+t]
            a3 = patt.tile([128, 128], f32, name="a3")
            for b in range(BL):
                for k in range(4):
                    nc.tensor.matmul(
                        a3[:, b * 8 : (b + 1) * 8],
                        hsT[:, k, b * 128 : (b + 1) * 128],
                        ttT[:, k, b * 8 : (b + 1) * 8],
                        start=(k == 0),
                        stop=(k == 3),
                    )
            expA = apool.tile([128, 128], f32, name="expA")
            nc.scalar.activation(expA, a3, AF.Exp, scale=4.0)
            expA_v = expA.rearrange("p (b t) -> p b t", t=LT)
            # row softmax (over t) then mean over s, divided by col sums (over s)
            rsum = apool.tile([128, BL], f32, name="rsum")
            nc.vector.tensor_reduce(rsum, expA_v, axis=mybir.AxisListType.X, op=ALU.add)
            rr = apool.tile([128, BL], f32, name="rr")
            nc.vector.reciprocal(rr, rsum)
            rnorm = apool.tile([128, 128], f32, name="rnorm")
            rr_b = bass.AP(tensor=rr.tensor, offset=rr.offset, ap=list(rr.ap) + [[0, LT]])
            nc.vector.tensor_tensor(rnorm.rearrange("p (b t) -> p b t", t=LT), expA_v, rr_b, op=ALU.mult)
            rvp = patt.tile([1, 128], f32, name="rvp")
            nc.tensor.matmul(rvp, ones[:, 0:1], rnorm, start=True, stop=True)
            csum = patt.tile([1, 128], f32, name="csum")
            nc.tensor.matmul(csum, ones[:, 0:1], expA, start=True, stop=True)
            rc = apool.tile([1, 128], f32, name="rc")
            nc.vector.reciprocal(rc, csum)
            q = apool.tile([1, 128], f32, name="q")
            nc.vector.scalar_tensor_tensor(q, rvp, 1.0 / LS, rc, op0=ALU.mult, op1=ALU.mult)
            qbc = patt.tile([128, 128], f32, name="qbc")
            nc.tensor.matmul(qbc, ones[0:1, :], q, start=True, stop=True)
            attw = apool.tile([128, 128], f32, name="attw")
            nc.vector.tensor_tensor(attw, expA, qbc, op=ALU.mult)
            attnT = apool.tile([128, BL], f32, name="attnT")
            nc.vector.tensor_reduce(attnT, attw.rearrange("p (b t) -> p b t", t=LT), axis=mybir.AxisListType.X, op=ALU.add)
            attnb = apool.tile([128, BL], bf16, name="attnb")
            nc.vector.tensor_copy(attnb, attnT)

            # de-transpose sen_h for the score contraction
            sen_h = apool.tile([128, BL, 4 * 128], bf16, name="sen_h")
            for b in range(BL):
                for k in range(4):
                    nc.sync.dma_start_transpose(
                        sen_h[:, b, k * 128 : (k + 1) * 128], hsT[:, k, b * 128 : (b + 1) * 128]
                    )
            scoT = patt.tile([128, 4 * BL], f32, name="scoT")
            for b in range(BL):
                for mh in range(4):
                    nc.tensor.matmul(
                        scoT[:, b * 4 + mh : b * 4 + mh + 1],
                        sen_h[:, b, mh * 128 : (mh + 1) * 128],
                        attnb[:, b : b + 1],
                        start=True,
                        stop=True,
                    )
            scoB = apool.tile([128, 4 * BL], bf16, name="scoB")
            nc.scalar.activation(scoB, scoT, AF.Copy)
            lgT = patt.tile([OUT, BL], f32, name="lgT")
            for mh in range(4):
                nc.tensor.matmul(
                    lgT, wout_sb[:, mh, :], scoB[:, mh :: 4], start=(mh == 0), stop=(mh == 3)
                )
            lgsb = apool.tile([OUT, BL], f32, name="lgsb")
            nc.scalar.activation(lgsb, lgT, AF.Identity, bias=bout_sb[0:OUT, 0:1])
            lg2 = patt.tile([BL, OUT], f32, name="lg2")
            nc.tensor.transpose(lg2, lgsb, ident[0:OUT, 0:OUT])
            eo = apool.tile([BL, OUT], f32, name="eo")
            nc.scalar.activation(eo, lg2, AF.Exp)
            es = apool.tile([BL, 1], f32, name="es")
            nc.vector.tensor_reduce(es, eo, axis=mybir.AxisListType.X, op=ALU.add)
            er = apool.tile([BL, 1], f32, name="er")
            nc.vector.reciprocal(er, es)
            res = apool.tile([BL, OUT], f32, name="res")
            nc.vector.tensor_scalar(res, eo, er, None, op0=ALU.mult)
            nc.sync.dma_start(d_out, res)

    _split_sync_waits(nc)
    return nc


_CACHE = {}


def _get_program():
    if "nc" not in _CACHE:
        _CACHE["nc"] = _build_program()
    return _CACHE["nc"]


def prepare_in_maps(inputs):
    """Host-side prep: shard + repack inputs into per-core in_maps."""
    bf = ml_dtypes.bfloat16
    sen = np.asarray(inputs["sentence_source"]).astype(np.int32)  # [B, LS]
    tgt = np.asarray(inputs["target_source"]).astype(np.int32)  # [B, LT]
    emb = np.ascontiguousarray(np.asarray(inputs["emb_W"], dtype=np.float32))
    emb = emb.copy()
    emb[0, :] = 0.0  # padding_idx

    def pack_wih(nm):
        W = np.asarray(inputs[f"Wih_{nm}"], dtype=np.float32).T.copy()  # [300, 1024]
        W[:, 2 * H : 3 * H] *= 2.0  # g-gate: tanh via sigmoid
        pack = np.zeros((3, 128, G4), dtype=bf)
        pack[0] = W[0:128].astype(bf)
        pack[1] = W[128:256].astype(bf)
        pack[2, 0:44] = W[256:300].astype(bf)
        return pack

    def pack_bias(nm):
        bias = (
            np.asarray(inputs[f"bih_{nm}"], dtype=np.float32)
            + np.asarray(inputs[f"bhh_{nm}"], dtype=np.float32)
        ).copy()
        bias[2 * H : 3 * H] *= 2.0
        return np.ascontiguousarray(bias.reshape(8, 128).T)

    def pack_whh(nm):
        W = np.asarray(inputs[f"Whh_{nm}"], dtype=np.float32).T.copy()  # [256, 1024]
        W *= 2.0  # hidden states are stored halved
        W[:, 2 * H : 3 * H] *= 2.0
        return np.ascontiguousarray(W.reshape(2, 128, G4).astype(bf))

    shared = {"emb": emb}
    for nm in ("sf", "sb", "tf", "tb"):
        shared[f"wih_{nm}"] = pack_wih(nm)
        shared[f"bias_{nm}"] = pack_bias(nm)
        shared[f"whh_{nm}"] = pack_whh(nm)
    Wout = np.asarray(inputs["Wout"], dtype=np.float32) * 2.0  # [3, 512]; sen_h halved
    shared["woutT"] = np.ascontiguousarray(Wout.T.reshape(4, 128, OUT).astype(bf))
    shared["boutT"] = np.asarray(inputs["bout"], dtype=np.float32).reshape(OUT, 1)

    in_maps = []
    for c in range(NCORES):
        sl = slice(c * BL, (c + 1) * BL)
        m = dict(shared)
        m["sen_idx"] = np.ascontiguousarray(sen[sl].reshape(NTOK_S // 128, 128).T)
        m["tgt_idx"] = np.ascontiguousarray(tgt[sl].reshape(NTOK_T // 128, 128).T)
        in_maps.append(m)
    return in_maps


def kernel(**inputs) -> np.ndarray:
    nc = _get_program()
    in_maps = prepare_in_maps(inputs)
    r = run_bass_kernel_spmd(nc, in_maps, core_ids=list(range(NCORES)))
    return np.concatenate([r.results[c]["out"] for c in range(NCORES)], axis=0)


if __name__ == "__main__":
    print("building program...")
    nc = _get_program()
    print("build OK")
